# revision 19
# baseline (speedup 1.0000x reference)
"""Trainium2 Bass kernel for the segment_reduce loss (nn_Loss_65996467471179).

Strategy (data-parallel over curves, fp16 streaming):
  - C=65536 curves of L=256 points. Shard curves across 8 cores (8192 each).
  - The five big N-length arrays (An, A_r, Ac, Aj, Ap) are downcast to fp16
    on the host inside kernel() (Ap additionally pre-divided by 1.1 so the
    crossover term needs no scalar multiply on device); each core streams its
    20MB fp16 shard once from HBM in [128, 4096] chunks (16 curves per
    partition row), computes all per-curve and global reductions on-chip in
    fp32 accumulators, and writes a small [128, ACCW] float32 accumulator
    block back to DRAM.
  - fp16 quantization perturbs the loss by ~2e-4 relative (dominant term is
    sum relu(-Ap) ~ 6.7e6; argmin tie-flips are unbiased) vs the 2e-2 gate.
  - Ci is only read at end-of-curve indices; that gather plus all C-length /
    O(4)-length pure-input terms (correlation moments, Rd25/dHa/Topt sign
    penalties) are folded on the host, which also combines the 8 cores'
    partial blocks into the final scalar in float64.

Engine assignment per chunk (curve rows along the free axis, J=16/partition):
  DVE:  d = An-Ar and Acj = Ac-Aj (TT, fp16 2x); per-curve sAcj/sAbs via
        4-stage pairwise TT-add trees + 3D tensor_reduce; per-curve min via
        4-stage TT-min tree + 3D reduce (f32 out); G = Aj - Ap' (TT, 2x);
        gint via J sliced stt (A==mn)*G with per-curve accum.
  ACT:  A = |Acj| (Abs); Square(d) accum -> MSE; Relu(-Ap') accum -> apn.
  GPSIMD: end-of-curve strided extracts + most of the tiny epilogue.
Findings baked in: TensorTensorReduce crashes HW; abs_max invalid on DVE;
fp8 streaming measured slower in this environment despite lower traffic;
GPSIMD TT is ~4x slower than DVE TT so both big subtracts live on DVE.
Measured: ~45us/exec quiet device (DMA floor ~39us at ~516GB/s/core);
~97-98us under heavy co-tenant HBM contention (baseline f32: 77.5us quiet /
~139-144us contended). 4-stage trees beat 2-stage by ~2.6us (the 1x-mode 3D
reduce shrinks 4x; stages 5-6 regress on instruction overhead).
Relative error vs the f32 jax reference: 1.9e-04.
"""

import os
import sys

import numpy as np

sys.path.insert(0, "/opt/trn_rl_repo")

import concourse.bass as bass
import concourse.bacc as bacc
import concourse.tile as tile
from concourse import mybir
from concourse.bass_utils import run_bass_kernel_spmd
from contextlib import ExitStack

NCORES = 8
C = 65536
L = 256
N = C * L
S = C // NCORES          # curves per core
NSH = S * L              # elements per core per big array
P = 128                  # partitions

KELVIN = 273.15
FIT_AP_CI = 500.0
TARGET_R = 0.7

f16 = mybir.dt.float16
f32 = mybir.dt.float32
f8 = mybir.dt.float8e4


VARIANT = dict(
    F=4096,             # elements per partition per chunk
    inp_bufs=2,
    wrk_bufs=2,
    d_on_pool=False,    # An-A_r subtract on GPSIMD (else DVE)
    g_on_pool=False,    # G = 1.1*Aj - Ap on GPSIMD (else DVE)
    sum_tree=True,      # per-curve sAcj/sAbs via TT-add trees (else sliced ts)
    sum_tree_stages=4,  # TT-add stages before the 3D add reduce
    no_G=False,         # skip G; per-curve select Aj and Ap separately
    ap_scale=True,      # host uploads Ap/1.1 so G is a plain TT subtract (2x)
    gint_tree=False,    # gint via 4x ts is_equal mask + TT mult + sum tree
    apn_on_act=True,    # sum relu(-Ap) on ACT (else DVE tensor_scalar)
    mse_on_act=True,    # d^2 sum on ACT Square (always)
    ttr_acj=True,       # fused sliced TTR for Acj+sAcj (else TT + sliced ts)
    tree_stages=4,      # pairwise TT-min stages before the 3D min reduce
    ends_on_pool=True,  # end-of-curve extracts on GPSIMD (else DVE)
    epi_on_pool=True,   # epilogue tensor_tensor ops on GPSIMD (else DVE)
    dma_acj_first=True,
    dma_spread=False,   # issue An/Ar input DMAs from the ACT DGE queue
    stat_bufs=2,        # double-buffer per-curve stat tiles across reps
                        # iterations (breaks WAR chains at the epilogue)
    junk_psum=False,    # ACT junk outputs in PSUM: scheduler issue, keep off
    fp8=(),             # big arrays streamed as fp8e4m3 instead of fp16
                        # (measured slower: GPSIMD software fp8 decode)
)


def _geom(v):
    F = v["F"]
    J = F // L
    M = NSH // (P * F)
    NCOL = M * J         # 64 for any F
    MSE0 = 0
    APN0 = MSE0 + M
    P30 = APN0 + M
    LS0 = P30 + NCOL
    E10 = LS0 + NCOL
    E20 = E10 + NCOL
    ACCW = E20 + NCOL
    return F, J, M, NCOL, MSE0, APN0, P30, LS0, E10, E20, ACCW


def _build_kernel(reps=None, variant=None):
    """reps=None: normal single-pass kernel. reps=R: wrap the whole body in a
    runtime For_i loop executing it R times (for HW timing via slope)."""
    v = dict(VARIANT)
    if variant:
        v.update(variant)
    F, J, M, NCOL, MSE0, APN0, P30, LS0, E10, E20, ACCW = _geom(v)

    nc = bacc.Bacc("TRN2", target_bir_lowering=False, debug=False, num_devices=NCORES)
    big = {
        nm: nc.declare_dram_parameter(
            nm, [NSH], f8 if nm in v["fp8"] else f16, isOutput=False)
        for nm in ("An", "Ar", "Ac", "Aj", "Ap")
    }
    wdev = nc.declare_dram_parameter("wdev", [P, NCOL], f32, isOutput=False)
    fitw = nc.declare_dram_parameter("fitw", [P, NCOL], f32, isOutput=False)
    acc = nc.declare_dram_parameter("acc", [P, ACCW], f32, isOutput=True)

    with ExitStack() as ctx:
        tc = ctx.enter_context(tile.TileContext(nc))
        inp = ctx.enter_context(tc.tile_pool(name="inp", bufs=v["inp_bufs"]))
        wrk = ctx.enter_context(tc.tile_pool(name="wrk", bufs=v["wrk_bufs"]))
        per = ctx.enter_context(tc.tile_pool(name="per", bufs=1))

        sta = ctx.enter_context(tc.tile_pool(name="sta", bufs=v["stat_bufs"]))
        g = {}
        for nm, shp, dt in (
            ("wT", [P, NCOL], f32), ("fT", [P, NCOL], f32),
            ("b8", [P, 1], f32),
        ):
            g[nm] = per.tile(shp, dt, tag=nm, name=nm)
        nc.vector.memset(g["b8"], 8.0)
        nc.sync.dma_start(out=g["wT"], in_=wdev[:])
        nc.sync.dma_start(out=g["fT"], in_=fitw[:])

        def body():
            gg = dict(g)
            for nm, shp, dt in (
                ("accT", [P, ACCW], f32), ("mnB", [P, NCOL], f32),
                ("sAcj", [P, NCOL], f32), ("sAbs", [P, NCOL], f32),
                ("gint", [P, NCOL], f32), ("eAp", [P, NCOL], f32),
                ("eAj", [P, NCOL], f32), ("eAc", [P, NCOL], f32),
                ("gAj", [P, NCOL], f32), ("gAp", [P, NCOL], f32),
                ("t1", [P, NCOL], f32), ("t2", [P, NCOL], f32),
                ("r1", [P, NCOL], f32), ("r2", [P, NCOL], f32),
            ):
                gg[nm] = sta.tile(shp, dt, tag=nm, name=nm)
            _trace_body(nc, tc, big, acc, inp, wrk, gg, v)

        if reps is None:
            body()
        else:
            with tc.For_i(0, reps, 1):
                body()

    nc.compile()
    return nc


def _trace_body(nc, tc, big, acc, inp, wrk, g, v):
    OP = mybir.AluOpType
    AF = mybir.ActivationFunctionType
    AX = mybir.AxisListType
    F, J, M, NCOL, MSE0, APN0, P30, LS0, E10, E20, ACCW = _geom(v)
    accT = g["accT"]

    for m in range(M):
        t = {}
        dma_order = ("Ac", "Aj", "Ap", "An", "Ar") if v["dma_acj_first"] \
            else ("An", "Ar", "Ac", "Aj", "Ap")
        for nm in dma_order:
            dt_nm = f8 if nm in v["fp8"] else f16
            t[nm] = inp.tile([P, F], dt_nm, tag=nm, name=f"in_{nm}_{m}")
            src = big[nm][:].rearrange("(m p f) -> m p f", m=M, p=P, f=F)[m]
            eng = nc.scalar if (v["dma_spread"] and nm in ("An", "Ar")) \
                else nc.sync
            eng.dma_start(out=t[nm], in_=src)

        cols = slice(m * J, (m + 1) * J)

        # --- GPSIMD: An-Ar subtract + end-of-curve extracts ---
        d = wrk.tile([P, F], f16, tag="d")
        d_eng = nc.gpsimd if v["d_on_pool"] else nc.vector
        d_eng.tensor_tensor(out=d, in0=t["An"], in1=t["Ar"], op=OP.subtract)
        ends_eng = nc.gpsimd if v["ends_on_pool"] else nc.vector
        for nm, dst in (("Ap", g["eAp"]), ("Aj", g["eAj"]), ("Ac", g["eAc"])):
            ends = t[nm].rearrange("p (j l) -> p j l", l=L)[:, :, L - 1 : L]
            ends_eng.tensor_copy(out=dst[:, cols], in_=ends)

        # --- ACT: global accumulations ---
        junk1 = wrk.tile([P, F], f16, tag="junk1",
                         space="PSUM" if v["junk_psum"] else None)
        nc.scalar.activation(
            out=junk1, in_=d, func=AF.Square,
            accum_out=accT[:, MSE0 + m : MSE0 + m + 1],
        )
        apn_dst = accT[:, APN0 + m : APN0 + m + 1]
        if v["apn_on_act"]:
            nc.scalar.activation(
                out=junk1, in_=t["Ap"], func=AF.Relu, scale=-1.0,
                accum_out=apn_dst,
            )
        else:
            # accum = sum(min(Ap, 0)) = -sum(relu(-Ap)); negated on host.
            nc.vector.tensor_scalar(
                out=junk1, in0=t["Ap"], scalar1=0.0, scalar2=None,
                op0=OP.min, op1=OP.add, accum_out=apn_dst,
            )

        # --- DVE: Acj = Ac - Aj; A = |Acj| on ACT ---
        Acj = wrk.tile([P, F], f16, tag="Acj")
        nc.vector.tensor_tensor(out=Acj, in0=t["Ac"], in1=t["Aj"],
                                op=OP.subtract)
        A = wrk.tile([P, F], f16, tag="A")
        nc.scalar.activation(out=A, in_=Acj, func=AF.Abs)

        def curve_sum(src, dst_cols, tagp):
            # per-curve sum over L of src [P, F] into dst_cols [P, J] f32
            if v["sum_tree"]:
                cur3, ln = src.rearrange("p (j l) -> p j l", l=L), L
                for st in range(v["sum_tree_stages"]):
                    half = ln // 2
                    Ts = wrk.tile([P, J * half], f16, tag=f"tre{st}")
                    Ts3 = Ts.rearrange("p (j h) -> p j h", h=half)
                    nc.vector.tensor_tensor(
                        out=Ts3, in0=cur3[:, :, 0:half],
                        in1=cur3[:, :, half:ln], op=OP.add)
                    cur3, ln = Ts3, half
                nc.vector.tensor_reduce(out=dst_cols, in_=cur3, axis=AX.X,
                                        op=OP.add)
            else:
                junk3 = wrk.tile([P, L], f16, tag=f"{tagp}jk")
                for j in range(J):
                    sl = slice(j * L, (j + 1) * L)
                    nc.vector.tensor_scalar(
                        out=junk3, in0=src[:, sl], scalar1=1.0, scalar2=None,
                        op0=OP.mult, op1=OP.add,
                        accum_out=dst_cols[:, j : j + 1],
                    )

        if v["sum_tree"]:
            curve_sum(Acj, g["sAcj"][:, cols], "sj")
            curve_sum(A, g["sAbs"][:, cols], "sb")
        else:
            curve_sum(Acj, g["sAcj"][:, m * J : (m + 1) * J], "sj")
            curve_sum(A, g["sAbs"][:, m * J : (m + 1) * J], "sb")

        # --- DVE: per-curve min via pairwise TT-min tree + 3D reduce ---
        A3 = A.rearrange("p (j l) -> p j l", l=L)
        stages = v["tree_stages"]
        cur3, ln = A3, L
        for s in range(stages):
            half = ln // 2
            Ts = wrk.tile([P, J * half], f16, tag=f"tre{s}" if s < 2
                          else f"T{s}")
            Ts3 = Ts.rearrange("p (j h) -> p j h", h=half)
            nc.vector.tensor_tensor(
                out=Ts3, in0=cur3[:, :, 0:half], in1=cur3[:, :, half:ln],
                op=OP.min,
            )
            cur3, ln = Ts3, half
        nc.vector.tensor_reduce(out=g["mnB"][:, cols], in_=cur3, axis=AX.X,
                                op=OP.min)

        # --- gint: select at argmin ---
        junkD = wrk.tile([P, L], f16, tag="junkD")
        if v["no_G"]:
            for j in range(J):
                c = m * J + j
                sl = slice(j * L, (j + 1) * L)
                nc.vector.scalar_tensor_tensor(
                    out=junkD, in0=A[:, sl], scalar=g["mnB"][:, c : c + 1],
                    in1=t["Aj"][:, sl], op0=OP.is_equal, op1=OP.mult,
                    accum_out=g["gAj"][:, c : c + 1],
                )
                nc.vector.scalar_tensor_tensor(
                    out=junkD, in0=A[:, sl], scalar=g["mnB"][:, c : c + 1],
                    in1=t["Ap"][:, sl], op0=OP.is_equal, op1=OP.mult,
                    accum_out=g["gAp"][:, c : c + 1],
                )
        else:
            G = wrk.tile([P, F], f16, tag="G")
            if v["ap_scale"]:
                # Ap was pre-divided by 1.1 on host: G' = Aj - Ap/1.1,
                # true G = 1.1*G' (scale folded into the epilogue relu)
                g_eng = nc.gpsimd if v["g_on_pool"] else nc.vector
                g_eng.tensor_tensor(out=G, in0=t["Aj"], in1=t["Ap"],
                                    op=OP.subtract)
            elif v["g_on_pool"]:
                # Pool has no stt: ACT scales 1.1*Aj, Pool TT subtracts Ap
                Aj11 = wrk.tile([P, F], f16, tag="Aj11")
                nc.scalar.activation(out=Aj11, in_=t["Aj"], func=AF.Identity,
                                     scale=1.1)
                nc.gpsimd.tensor_tensor(out=G, in0=Aj11, in1=t["Ap"],
                                        op=OP.subtract)
            else:
                nc.vector.scalar_tensor_tensor(
                    out=G, in0=t["Aj"], scalar=1.1, in1=t["Ap"],
                    op0=OP.mult, op1=OP.subtract,
                )
            if v["gint_tree"]:
                mask = wrk.tile([P, F], f16, tag="mask")
                for j in range(J):
                    c = m * J + j
                    sl = slice(j * L, (j + 1) * L)
                    nc.vector.tensor_scalar(
                        out=mask[:, sl], in0=A[:, sl],
                        scalar1=g["mnB"][:, c : c + 1], scalar2=None,
                        op0=OP.is_equal, op1=OP.bypass,
                    )
                nc.vector.tensor_tensor(out=mask, in0=mask, in1=G,
                                        op=OP.mult)
                curve_sum(mask, g["gint"][:, cols], "gi")
            else:
                for j in range(J):
                    c = m * J + j
                    sl = slice(j * L, (j + 1) * L)
                    nc.vector.scalar_tensor_tensor(
                        out=junkD, in0=A[:, sl], scalar=g["mnB"][:, c : c + 1],
                        in1=G[:, sl], op0=OP.is_equal, op1=OP.mult,
                        accum_out=g["gint"][:, c : c + 1],
                    )

    # --- epilogue on [128, NCOL] column blocks ---
    AF = mybir.ActivationFunctionType
    OP = mybir.AluOpType
    epi = nc.gpsimd if v["epi_on_pool"] else nc.vector
    t1, t2, r1, r2 = g["t1"], g["t2"], g["r1"], g["r2"]
    sAbs, sAcj, b8 = g["sAbs"], g["sAcj"], g["b8"]
    # ls penalty: relu(8-ls_Aj)+relu(8-ls_Ac), ls_* = (sAbs -+ sAcj)/2
    epi.tensor_tensor(out=t1, in0=sAbs, in1=sAcj, op=OP.add)
    nc.scalar.activation(out=r1, in_=t1, func=AF.Relu, scale=-0.5, bias=b8)
    epi.tensor_tensor(out=t2, in0=sAbs, in1=sAcj, op=OP.subtract)
    nc.scalar.activation(out=r2, in_=t2, func=AF.Relu, scale=-0.5, bias=b8)
    epi.tensor_tensor(out=t1, in0=r1, in1=r2, op=OP.add)
    epi.tensor_tensor(out=accT[:, LS0 : LS0 + NCOL], in0=t1, in1=g["wT"],
                      op=OP.mult)
    # crossover penalty: 3*relu(gint) == relu(3*gint)
    p3_scale = 3.0 * 1.1 if v["ap_scale"] else 3.0
    if v["no_G"]:
        aj_s = 1.0 if v["ap_scale"] else 1.1
        nc.vector.scalar_tensor_tensor(
            out=g["gint"], in0=g["gAj"], scalar=aj_s, in1=g["gAp"],
            op0=OP.mult, op1=OP.subtract)
        if not v["ap_scale"]:
            p3_scale = 3.0
    nc.scalar.activation(out=accT[:, P30 : P30 + NCOL], in_=g["gint"],
                         func=AF.Relu, scale=p3_scale)
    # end-of-curve penalties
    if v["ap_scale"]:
        # eAp holds Ap/1.1 at curve ends; restore the true value
        nc.scalar.activation(out=g["eAp"], in_=g["eAp"], func=AF.Identity,
                             scale=1.1)
    epi.tensor_tensor(out=t2, in0=g["eAp"], in1=g["eAj"], op=OP.subtract)
    nc.scalar.activation(out=r1, in_=t2, func=AF.Relu)
    epi.tensor_tensor(out=accT[:, E10 : E10 + NCOL], in0=r1, in1=g["fT"],
                      op=OP.mult)
    epi.tensor_tensor(out=t2, in0=g["eAj"], in1=g["eAc"], op=OP.subtract)
    nc.scalar.activation(out=accT[:, E20 : E20 + NCOL], in_=t2, func=AF.Relu)

    nc.sync.dma_start(out=acc[:], in_=accT)


_NC_CACHE = {}
LAST_RESULTS = None


def _get_nc(reps=None, variant=None):
    key = (reps, tuple(sorted((variant or {}).items())))
    if key not in _NC_CACHE:
        _NC_CACHE[key] = _build_kernel(reps, variant)
    return _NC_CACHE[key]


def _curve_layout(x_per_curve: np.ndarray, v=None) -> np.ndarray:
    """Map a per-curve [S] array for one core into the device [P, NCOL] layout:
    dev[p, m*J + j] corresponds to curve m*(P*J) + p*J + j."""
    F, J, M, NCOL = _geom(v or VARIANT)[:4]
    return np.ascontiguousarray(
        x_per_curve.reshape(M, P, J).transpose(1, 0, 2).reshape(P, NCOL)
    )


def prep_in_maps(An_o, Ac_o, Aj_o, Ap_o, A_r, Ci, mask_lightresp, v=None):
    w_full = (mask_lightresp == 0).astype(np.float32)        # [C]
    Ci_end = np.ascontiguousarray(Ci[L - 1 :: L])            # [C]
    fit_full = ((Ci_end > FIT_AP_CI).astype(np.float32) * w_full)  # [C]

    import ml_dtypes
    vv = v or VARIANT
    def h(x, nm):
        if nm in vv["fp8"]:
            return np.ascontiguousarray(
                x.astype(ml_dtypes.float8_e4m3)).view(np.uint8)
        return np.ascontiguousarray(x, dtype=np.float16)
    in_maps = []
    ap_src = Ap_o * np.float32(1.0 / 1.1) if vv["ap_scale"] else Ap_o
    for k in range(NCORES):
        cur = slice(k * S, (k + 1) * S)
        el = slice(k * NSH, (k + 1) * NSH)
        in_maps.append({
            "An": h(An_o[el], "An"),
            "Ar": h(A_r[el], "Ar"),
            "Ac": h(Ac_o[el], "Ac"),
            "Aj": h(Aj_o[el], "Aj"),
            "Ap": h(ap_src[el], "Ap"),
            "wdev": _curve_layout(w_full[cur], v),
            "fitw": _curve_layout(fit_full[cur], v),
        })
    return in_maps


def kernel(An_o, Ac_o, Aj_o, Ap_o, A_r, Ci, Vcmax25, Jmax25, Rd25,
           dHa_Vcmax, dHa_Jmax, dHa_TPU, Topt_Vcmax, Topt_Jmax, Topt_TPU,
           mask_lightresp):
    An_o, Ac_o, Aj_o, Ap_o, A_r, Ci = (
        np.asarray(x) for x in (An_o, Ac_o, Aj_o, Ap_o, A_r, Ci))
    (Vcmax25, Jmax25, Rd25, dHa_Vcmax, dHa_Jmax, dHa_TPU,
     Topt_Vcmax, Topt_Jmax, Topt_TPU, mask_lightresp) = (
        np.asarray(x) for x in (Vcmax25, Jmax25, Rd25, dHa_Vcmax, dHa_Jmax,
                                dHa_TPU, Topt_Vcmax, Topt_Jmax, Topt_TPU,
                                mask_lightresp))
    v = dict(VARIANT)
    F, J, M, NCOL, MSE0, APN0, P30, LS0, E10, E20, ACCW = _geom(v)
    nc = _get_nc()
    in_maps = prep_in_maps(An_o, Ac_o, Aj_o, Ap_o, A_r, Ci, mask_lightresp, v)

    try:
        res = run_bass_kernel_spmd(
            nc, in_maps, core_ids=list(range(NCORES)),
            trace=bool(int(os.environ.get("KERNEL_TRACE", "0"))),
        )
    except ModuleNotFoundError:
        os.environ["BASS_NEVER_TRACE"] = "1"
        res = run_bass_kernel_spmd(nc, in_maps, core_ids=list(range(NCORES)))
    global LAST_RESULTS
    LAST_RESULTS = res
    blocks = [r["acc"].astype(np.float64) for r in res.results]

    mse = sum(b[:, MSE0 : MSE0 + M].sum() for b in blocks)
    apn = sum(b[:, APN0 : APN0 + M].sum() for b in blocks)
    p3 = sum(b[:, P30 : P30 + NCOL].sum() for b in blocks)
    ls = sum(b[:, LS0 : LS0 + NCOL].sum() for b in blocks)
    e1 = sum(b[:, E10 : E10 + NCOL].sum() for b in blocks)
    e2 = sum(b[:, E20 : E20 + NCOL].sum() for b in blocks)
    if not v["apn_on_act"]:
        apn = -apn
    if v["ap_scale"]:
        apn *= 1.1

    # host-side terms (tiny inputs only)
    w = (mask_lightresp == 0).astype(np.float64)
    x = Jmax25.astype(np.float64)
    y = Vcmax25.astype(np.float64)
    nw = w.sum()
    if nw > 0:
        my = (w * y).sum() / nw
        mx = (w * x).sum() / nw
        vy = (y - my) * w
        vx = (x - mx) * w
        denom = np.sqrt((vx * vx).sum()) * np.sqrt((vy * vy).sum())
        cost = (vx * vy).sum() / denom if denom != 0.0 else np.nan
    else:
        cost = np.nan
    if np.isnan(cost):
        cost = 0.0
    cost = min(cost, TARGET_R)

    relu = lambda z: np.maximum(z, 0.0)
    loss = mse * 10.0 / N
    loss += TARGET_R - cost
    loss += relu(-Rd25.astype(np.float64)).sum()
    loss += relu(-dHa_Vcmax.astype(np.float64)).sum() * 10.0
    loss += relu(-dHa_Jmax.astype(np.float64)).sum()
    loss += relu(-dHa_TPU.astype(np.float64)).sum()
    loss += relu(KELVIN - Topt_Vcmax.astype(np.float64)).sum()
    loss += relu(KELVIN - Topt_Jmax.astype(np.float64)).sum()
    loss += relu(KELVIN - Topt_TPU.astype(np.float64)).sum()
    loss += apn
    loss += e1 * 0.15
    loss += e2
    loss += p3
    loss += ls

    return np.asarray(loss, dtype=np.float32)


# revision 23
# speedup vs baseline: 1.0337x; 1.0337x over previous
"""Trainium2 Bass kernel for the segment_reduce loss (nn_Loss_65996467471179).

Strategy (data-parallel over curves, fp16 streaming):
  - C=65536 curves of L=256 points. Shard curves across 8 cores (8192 each).
  - The five big N-length arrays (An, A_r, Ac, Aj, Ap) are downcast to fp16
    on the host inside kernel() (Ap additionally pre-divided by 1.1 so the
    crossover term needs no scalar multiply on device); each core streams its
    20MB fp16 shard once from HBM in [128, 4096] chunks (16 curves per
    partition row), computes all per-curve and global reductions on-chip in
    fp32 accumulators, and writes a small [128, ACCW] float32 accumulator
    block back to DRAM.
  - fp16 quantization perturbs the loss by ~2e-4 relative (dominant term is
    sum relu(-Ap) ~ 6.7e6; argmin tie-flips are unbiased) vs the 2e-2 gate.
  - Ci is only read at end-of-curve indices; that gather plus all C-length /
    O(4)-length pure-input terms (correlation moments, Rd25/dHa/Topt sign
    penalties) are folded on the host, which also combines the 8 cores'
    partial blocks into the final scalar in float64.

Engine assignment per chunk (curve rows along the free axis, J=16/partition):
  DVE:  d = An-Ar and Acj = Ac-Aj (TT, fp16 2x); per-curve sAcj/sAbs via
        4-stage pairwise TT-add trees + 3D tensor_reduce; per-curve min via
        4-stage TT-min tree + 3D reduce (f32 out); G = Aj - Ap' (TT, 2x);
        gint via J sliced stt (A==mn)*G with per-curve accum.
  ACT:  A = |Acj| (Abs); Square(d) accum -> MSE; Relu(-Ap') accum -> apn.
  GPSIMD: end-of-curve strided extracts + most of the tiny epilogue.
Findings baked in: TensorTensorReduce crashes HW; abs_max invalid on DVE;
fp8 streaming measured slower in this environment despite lower traffic;
GPSIMD TT is ~4x slower than DVE TT so both big subtracts live on DVE.
Measured: ~45us/exec quiet device (DMA floor ~39us at ~516GB/s/core);
~97-98us under heavy co-tenant HBM contention (baseline f32: 77.5us quiet /
~139-144us contended). 4-stage trees beat 2-stage by ~2.6us (the 1x-mode 3D
reduce shrinks 4x; stages 5-6 regress on instruction overhead).
Relative error vs the f32 jax reference: 1.9e-04.
"""

import os
import sys

import numpy as np

sys.path.insert(0, "/opt/trn_rl_repo")

import concourse.bass as bass
import concourse.bacc as bacc
import concourse.tile as tile
from concourse import mybir
from concourse.bass_utils import run_bass_kernel_spmd
from contextlib import ExitStack

NCORES = 8
C = 65536
L = 256
N = C * L
S = C // NCORES          # curves per core
NSH = S * L              # elements per core per big array
P = 128                  # partitions

KELVIN = 273.15
FIT_AP_CI = 500.0
TARGET_R = 0.7

f16 = mybir.dt.float16
f32 = mybir.dt.float32
f8 = mybir.dt.float8e4


VARIANT = dict(
    F=4096,             # elements per partition per chunk
    inp_bufs=2,
    wrk_bufs=2,
    d_on_pool=False,    # An-A_r subtract on GPSIMD (else DVE)
    g_on_pool=False,    # G = 1.1*Aj - Ap on GPSIMD (else DVE)
    sum_tree=True,      # per-curve sAcj/sAbs via TT-add trees (else sliced ts)
    sum_tree_stages=4,  # TT-add stages before the 3D add reduce
    no_G=False,         # skip G; per-curve select Aj and Ap separately
    ap_scale=True,      # host uploads Ap/1.1 so G is a plain TT subtract (2x)
    gint_tree=False,    # gint via 4x ts is_equal mask + TT mult + sum tree
    apn_on_act=True,    # sum relu(-Ap) on ACT (else DVE tensor_scalar)
    mse_on_act=True,    # d^2 sum on ACT Square (always)
    ttr_acj=True,       # fused sliced TTR for Acj+sAcj (else TT + sliced ts)
    tree_stages=4,      # pairwise TT-min stages before the 3D min reduce
    ends_on_pool=True,  # end-of-curve extracts on GPSIMD (else DVE)
    epi_on_pool=True,   # epilogue tensor_tensor ops on GPSIMD (else DVE)
    dma_acj_first=True,
    dma_spread=False,   # issue An/Ar input DMAs from the ACT DGE queue
    stat_bufs=2,        # double-buffer per-curve stat tiles across reps
                        # iterations (breaks WAR chains at the epilogue)
    junk_psum=False,    # ACT junk outputs in PSUM: scheduler issue, keep off
    chunked_epi=False,  # per-chunk epilogue measured ~1.5us slower than
                        # trailing (stat_bufs=2 already pipelines the tail)
    fp8=(),             # big arrays streamed as fp8e4m3 instead of fp16
                        # (measured slower: GPSIMD software fp8 decode)
)


def _geom(v):
    F = v["F"]
    J = F // L
    M = NSH // (P * F)
    NCOL = M * J         # 64 for any F
    MSE0 = 0
    APN0 = MSE0 + M
    P30 = APN0 + M
    LS0 = P30 + NCOL
    E10 = LS0 + NCOL
    E20 = E10 + NCOL
    ACCW = E20 + NCOL
    return F, J, M, NCOL, MSE0, APN0, P30, LS0, E10, E20, ACCW


def _build_kernel(reps=None, variant=None):
    """reps=None: normal single-pass kernel. reps=R: wrap the whole body in a
    runtime For_i loop executing it R times (for HW timing via slope)."""
    v = dict(VARIANT)
    if variant:
        v.update(variant)
    F, J, M, NCOL, MSE0, APN0, P30, LS0, E10, E20, ACCW = _geom(v)

    nc = bacc.Bacc("TRN2", target_bir_lowering=False, debug=False, num_devices=NCORES)
    big = {
        nm: nc.declare_dram_parameter(
            nm, [NSH], f8 if nm in v["fp8"] else f16, isOutput=False)
        for nm in ("An", "Ar", "Ac", "Aj", "Ap")
    }
    wdev = nc.declare_dram_parameter("wdev", [P, NCOL], f32, isOutput=False)
    fitw = nc.declare_dram_parameter("fitw", [P, NCOL], f32, isOutput=False)
    acc = nc.declare_dram_parameter("acc", [P, ACCW], f32, isOutput=True)

    with ExitStack() as ctx:
        tc = ctx.enter_context(tile.TileContext(nc))
        inp = ctx.enter_context(tc.tile_pool(name="inp", bufs=v["inp_bufs"]))
        wrk = ctx.enter_context(tc.tile_pool(name="wrk", bufs=v["wrk_bufs"]))
        per = ctx.enter_context(tc.tile_pool(name="per", bufs=1))

        sta = ctx.enter_context(tc.tile_pool(name="sta", bufs=v["stat_bufs"]))
        g = {}
        for nm, shp, dt in (
            ("wT", [P, NCOL], f32), ("fT", [P, NCOL], f32),
            ("b8", [P, 1], f32),
        ):
            g[nm] = per.tile(shp, dt, tag=nm, name=nm)
        nc.vector.memset(g["b8"], 8.0)
        nc.sync.dma_start(out=g["wT"], in_=wdev[:])
        nc.sync.dma_start(out=g["fT"], in_=fitw[:])

        def body():
            gg = dict(g)
            for nm, shp, dt in (
                ("accT", [P, ACCW], f32), ("mnB", [P, NCOL], f32),
                ("sAcj", [P, NCOL], f32), ("sAbs", [P, NCOL], f32),
                ("gint", [P, NCOL], f32), ("eAp", [P, NCOL], f32),
                ("eAj", [P, NCOL], f32), ("eAc", [P, NCOL], f32),
                ("gAj", [P, NCOL], f32), ("gAp", [P, NCOL], f32),
                ("t1", [P, NCOL], f32), ("t2", [P, NCOL], f32),
                ("r1", [P, NCOL], f32), ("r2", [P, NCOL], f32),
            ):
                gg[nm] = sta.tile(shp, dt, tag=nm, name=nm)
            _trace_body(nc, tc, big, acc, inp, wrk, gg, v)

        if reps is None:
            body()
        else:
            with tc.For_i(0, reps, 1):
                body()

    nc.compile()
    return nc


def _trace_body(nc, tc, big, acc, inp, wrk, g, v):
    OP = mybir.AluOpType
    AF = mybir.ActivationFunctionType
    AX = mybir.AxisListType
    F, J, M, NCOL, MSE0, APN0, P30, LS0, E10, E20, ACCW = _geom(v)
    accT = g["accT"]

    def epilogue(lo, hi):
        W = hi - lo
        cs = slice(lo, hi)
        epi = nc.gpsimd if v["epi_on_pool"] else nc.vector
        t1, t2, r1, r2 = g["t1"], g["t2"], g["r1"], g["r2"]
        sAbs, sAcj, b8 = g["sAbs"], g["sAcj"], g["b8"]
        # ls penalty: relu(8-ls_Aj)+relu(8-ls_Ac), ls_* = (sAbs -+ sAcj)/2
        epi.tensor_tensor(out=t1[:, :W], in0=sAbs[:, cs], in1=sAcj[:, cs],
                          op=OP.add)
        nc.scalar.activation(out=r1[:, :W], in_=t1[:, :W], func=AF.Relu,
                             scale=-0.5, bias=b8)
        epi.tensor_tensor(out=t2[:, :W], in0=sAbs[:, cs], in1=sAcj[:, cs],
                          op=OP.subtract)
        nc.scalar.activation(out=r2[:, :W], in_=t2[:, :W], func=AF.Relu,
                             scale=-0.5, bias=b8)
        epi.tensor_tensor(out=t1[:, :W], in0=r1[:, :W], in1=r2[:, :W],
                          op=OP.add)
        epi.tensor_tensor(out=accT[:, LS0 + lo : LS0 + hi], in0=t1[:, :W],
                          in1=g["wT"][:, cs], op=OP.mult)
        # crossover penalty: 3*relu(gint) == relu(3*gint)
        p3_scale = 3.0 * 1.1 if v["ap_scale"] else 3.0
        if v["no_G"]:
            aj_s = 1.0 if v["ap_scale"] else 1.1
            nc.vector.scalar_tensor_tensor(
                out=g["gint"][:, cs], in0=g["gAj"][:, cs], scalar=aj_s,
                in1=g["gAp"][:, cs], op0=OP.mult, op1=OP.subtract)
        nc.scalar.activation(out=accT[:, P30 + lo : P30 + hi],
                             in_=g["gint"][:, cs], func=AF.Relu,
                             scale=p3_scale)
        # end-of-curve penalties
        if v["ap_scale"]:
            # eAp holds Ap/1.1 at curve ends; restore the true value
            nc.scalar.activation(out=g["eAp"][:, cs], in_=g["eAp"][:, cs],
                                 func=AF.Identity, scale=1.1)
        epi.tensor_tensor(out=t2[:, :W], in0=g["eAp"][:, cs],
                          in1=g["eAj"][:, cs], op=OP.subtract)
        nc.scalar.activation(out=r1[:, :W], in_=t2[:, :W], func=AF.Relu)
        epi.tensor_tensor(out=accT[:, E10 + lo : E10 + hi], in0=r1[:, :W],
                          in1=g["fT"][:, cs], op=OP.mult)
        epi.tensor_tensor(out=t2[:, :W], in0=g["eAj"][:, cs],
                          in1=g["eAc"][:, cs], op=OP.subtract)
        nc.scalar.activation(out=accT[:, E20 + lo : E20 + hi],
                             in_=t2[:, :W], func=AF.Relu)

    for m in range(M):
        t = {}
        dma_order = ("Ac", "Aj", "Ap", "An", "Ar") if v["dma_acj_first"] \
            else ("An", "Ar", "Ac", "Aj", "Ap")
        for nm in dma_order:
            dt_nm = f8 if nm in v["fp8"] else f16
            t[nm] = inp.tile([P, F], dt_nm, tag=nm, name=f"in_{nm}_{m}")
            src = big[nm][:].rearrange("(m p f) -> m p f", m=M, p=P, f=F)[m]
            eng = nc.scalar if (v["dma_spread"] and nm in ("An", "Ar")) \
                else nc.sync
            eng.dma_start(out=t[nm], in_=src)

        cols = slice(m * J, (m + 1) * J)

        # --- GPSIMD: An-Ar subtract + end-of-curve extracts ---
        d = wrk.tile([P, F], f16, tag="d")
        d_eng = nc.gpsimd if v["d_on_pool"] else nc.vector
        d_eng.tensor_tensor(out=d, in0=t["An"], in1=t["Ar"], op=OP.subtract)
        ends_eng = nc.gpsimd if v["ends_on_pool"] else nc.vector
        for nm, dst in (("Ap", g["eAp"]), ("Aj", g["eAj"]), ("Ac", g["eAc"])):
            ends = t[nm].rearrange("p (j l) -> p j l", l=L)[:, :, L - 1 : L]
            ends_eng.tensor_copy(out=dst[:, cols], in_=ends)

        # --- ACT: global accumulations ---
        junk1 = wrk.tile([P, F], f16, tag="junk1",
                         space="PSUM" if v["junk_psum"] else None)
        nc.scalar.activation(
            out=junk1, in_=d, func=AF.Square,
            accum_out=accT[:, MSE0 + m : MSE0 + m + 1],
        )
        apn_dst = accT[:, APN0 + m : APN0 + m + 1]
        if v["apn_on_act"]:
            nc.scalar.activation(
                out=junk1, in_=t["Ap"], func=AF.Relu, scale=-1.0,
                accum_out=apn_dst,
            )
        else:
            # accum = sum(min(Ap, 0)) = -sum(relu(-Ap)); negated on host.
            nc.vector.tensor_scalar(
                out=junk1, in0=t["Ap"], scalar1=0.0, scalar2=None,
                op0=OP.min, op1=OP.add, accum_out=apn_dst,
            )

        # --- DVE: Acj = Ac - Aj; A = |Acj| on ACT ---
        Acj = wrk.tile([P, F], f16, tag="Acj")
        nc.vector.tensor_tensor(out=Acj, in0=t["Ac"], in1=t["Aj"],
                                op=OP.subtract)
        A = wrk.tile([P, F], f16, tag="A")
        nc.scalar.activation(out=A, in_=Acj, func=AF.Abs)

        def curve_sum(src, dst_cols, tagp):
            # per-curve sum over L of src [P, F] into dst_cols [P, J] f32
            if v["sum_tree"]:
                cur3, ln = src.rearrange("p (j l) -> p j l", l=L), L
                for st in range(v["sum_tree_stages"]):
                    half = ln // 2
                    Ts = wrk.tile([P, J * half], f16, tag=f"tre{st}")
                    Ts3 = Ts.rearrange("p (j h) -> p j h", h=half)
                    nc.vector.tensor_tensor(
                        out=Ts3, in0=cur3[:, :, 0:half],
                        in1=cur3[:, :, half:ln], op=OP.add)
                    cur3, ln = Ts3, half
                nc.vector.tensor_reduce(out=dst_cols, in_=cur3, axis=AX.X,
                                        op=OP.add)
            else:
                junk3 = wrk.tile([P, L], f16, tag=f"{tagp}jk")
                for j in range(J):
                    sl = slice(j * L, (j + 1) * L)
                    nc.vector.tensor_scalar(
                        out=junk3, in0=src[:, sl], scalar1=1.0, scalar2=None,
                        op0=OP.mult, op1=OP.add,
                        accum_out=dst_cols[:, j : j + 1],
                    )

        if v["sum_tree"]:
            curve_sum(Acj, g["sAcj"][:, cols], "sj")
            curve_sum(A, g["sAbs"][:, cols], "sb")
        else:
            curve_sum(Acj, g["sAcj"][:, m * J : (m + 1) * J], "sj")
            curve_sum(A, g["sAbs"][:, m * J : (m + 1) * J], "sb")

        # --- DVE: per-curve min via pairwise TT-min tree + 3D reduce ---
        A3 = A.rearrange("p (j l) -> p j l", l=L)
        stages = v["tree_stages"]
        cur3, ln = A3, L
        for s in range(stages):
            half = ln // 2
            Ts = wrk.tile([P, J * half], f16, tag=f"tre{s}" if s < 2
                          else f"T{s}")
            Ts3 = Ts.rearrange("p (j h) -> p j h", h=half)
            nc.vector.tensor_tensor(
                out=Ts3, in0=cur3[:, :, 0:half], in1=cur3[:, :, half:ln],
                op=OP.min,
            )
            cur3, ln = Ts3, half
        nc.vector.tensor_reduce(out=g["mnB"][:, cols], in_=cur3, axis=AX.X,
                                op=OP.min)

        # --- gint: select at argmin ---
        junkD = wrk.tile([P, L], f16, tag="junkD")
        if v["no_G"]:
            for j in range(J):
                c = m * J + j
                sl = slice(j * L, (j + 1) * L)
                nc.vector.scalar_tensor_tensor(
                    out=junkD, in0=A[:, sl], scalar=g["mnB"][:, c : c + 1],
                    in1=t["Aj"][:, sl], op0=OP.is_equal, op1=OP.mult,
                    accum_out=g["gAj"][:, c : c + 1],
                )
                nc.vector.scalar_tensor_tensor(
                    out=junkD, in0=A[:, sl], scalar=g["mnB"][:, c : c + 1],
                    in1=t["Ap"][:, sl], op0=OP.is_equal, op1=OP.mult,
                    accum_out=g["gAp"][:, c : c + 1],
                )
        else:
            G = wrk.tile([P, F], f16, tag="G")
            if v["ap_scale"]:
                # Ap was pre-divided by 1.1 on host: G' = Aj - Ap/1.1,
                # true G = 1.1*G' (scale folded into the epilogue relu)
                g_eng = nc.gpsimd if v["g_on_pool"] else nc.vector
                g_eng.tensor_tensor(out=G, in0=t["Aj"], in1=t["Ap"],
                                    op=OP.subtract)
            elif v["g_on_pool"]:
                # Pool has no stt: ACT scales 1.1*Aj, Pool TT subtracts Ap
                Aj11 = wrk.tile([P, F], f16, tag="Aj11")
                nc.scalar.activation(out=Aj11, in_=t["Aj"], func=AF.Identity,
                                     scale=1.1)
                nc.gpsimd.tensor_tensor(out=G, in0=Aj11, in1=t["Ap"],
                                        op=OP.subtract)
            else:
                nc.vector.scalar_tensor_tensor(
                    out=G, in0=t["Aj"], scalar=1.1, in1=t["Ap"],
                    op0=OP.mult, op1=OP.subtract,
                )
            if v["gint_tree"]:
                mask = wrk.tile([P, F], f16, tag="mask")
                for j in range(J):
                    c = m * J + j
                    sl = slice(j * L, (j + 1) * L)
                    nc.vector.tensor_scalar(
                        out=mask[:, sl], in0=A[:, sl],
                        scalar1=g["mnB"][:, c : c + 1], scalar2=None,
                        op0=OP.is_equal, op1=OP.bypass,
                    )
                nc.vector.tensor_tensor(out=mask, in0=mask, in1=G,
                                        op=OP.mult)
                curve_sum(mask, g["gint"][:, cols], "gi")
            else:
                for j in range(J):
                    c = m * J + j
                    sl = slice(j * L, (j + 1) * L)
                    nc.vector.scalar_tensor_tensor(
                        out=junkD, in0=A[:, sl], scalar=g["mnB"][:, c : c + 1],
                        in1=G[:, sl], op0=OP.is_equal, op1=OP.mult,
                        accum_out=g["gint"][:, c : c + 1],
                    )

        if v["chunked_epi"]:
            epilogue(m * J, (m + 1) * J)

    if not v["chunked_epi"]:
        epilogue(0, NCOL)
    nc.sync.dma_start(out=acc[:], in_=accT)


_NC_CACHE = {}
LAST_RESULTS = None


def _get_nc(reps=None, variant=None):
    key = (reps, tuple(sorted((variant or {}).items())))
    if key not in _NC_CACHE:
        _NC_CACHE[key] = _build_kernel(reps, variant)
    return _NC_CACHE[key]


def _curve_layout(x_per_curve: np.ndarray, v=None) -> np.ndarray:
    """Map a per-curve [S] array for one core into the device [P, NCOL] layout:
    dev[p, m*J + j] corresponds to curve m*(P*J) + p*J + j."""
    F, J, M, NCOL = _geom(v or VARIANT)[:4]
    return np.ascontiguousarray(
        x_per_curve.reshape(M, P, J).transpose(1, 0, 2).reshape(P, NCOL)
    )


def prep_in_maps(An_o, Ac_o, Aj_o, Ap_o, A_r, Ci, mask_lightresp, v=None):
    w_full = (mask_lightresp == 0).astype(np.float32)        # [C]
    Ci_end = np.ascontiguousarray(Ci[L - 1 :: L])            # [C]
    fit_full = ((Ci_end > FIT_AP_CI).astype(np.float32) * w_full)  # [C]

    import ml_dtypes
    vv = v or VARIANT
    def h(x, nm):
        if nm in vv["fp8"]:
            return np.ascontiguousarray(
                x.astype(ml_dtypes.float8_e4m3)).view(np.uint8)
        return np.ascontiguousarray(x, dtype=np.float16)
    in_maps = []
    ap_src = Ap_o * np.float32(1.0 / 1.1) if vv["ap_scale"] else Ap_o
    for k in range(NCORES):
        cur = slice(k * S, (k + 1) * S)
        el = slice(k * NSH, (k + 1) * NSH)
        in_maps.append({
            "An": h(An_o[el], "An"),
            "Ar": h(A_r[el], "Ar"),
            "Ac": h(Ac_o[el], "Ac"),
            "Aj": h(Aj_o[el], "Aj"),
            "Ap": h(ap_src[el], "Ap"),
            "wdev": _curve_layout(w_full[cur], v),
            "fitw": _curve_layout(fit_full[cur], v),
        })
    return in_maps


def kernel(An_o, Ac_o, Aj_o, Ap_o, A_r, Ci, Vcmax25, Jmax25, Rd25,
           dHa_Vcmax, dHa_Jmax, dHa_TPU, Topt_Vcmax, Topt_Jmax, Topt_TPU,
           mask_lightresp):
    An_o, Ac_o, Aj_o, Ap_o, A_r, Ci = (
        np.asarray(x) for x in (An_o, Ac_o, Aj_o, Ap_o, A_r, Ci))
    (Vcmax25, Jmax25, Rd25, dHa_Vcmax, dHa_Jmax, dHa_TPU,
     Topt_Vcmax, Topt_Jmax, Topt_TPU, mask_lightresp) = (
        np.asarray(x) for x in (Vcmax25, Jmax25, Rd25, dHa_Vcmax, dHa_Jmax,
                                dHa_TPU, Topt_Vcmax, Topt_Jmax, Topt_TPU,
                                mask_lightresp))
    v = dict(VARIANT)
    F, J, M, NCOL, MSE0, APN0, P30, LS0, E10, E20, ACCW = _geom(v)
    nc = _get_nc()
    in_maps = prep_in_maps(An_o, Ac_o, Aj_o, Ap_o, A_r, Ci, mask_lightresp, v)

    try:
        res = run_bass_kernel_spmd(
            nc, in_maps, core_ids=list(range(NCORES)),
            trace=bool(int(os.environ.get("KERNEL_TRACE", "0"))),
        )
    except ModuleNotFoundError:
        os.environ["BASS_NEVER_TRACE"] = "1"
        res = run_bass_kernel_spmd(nc, in_maps, core_ids=list(range(NCORES)))
    global LAST_RESULTS
    LAST_RESULTS = res
    blocks = [r["acc"].astype(np.float64) for r in res.results]

    mse = sum(b[:, MSE0 : MSE0 + M].sum() for b in blocks)
    apn = sum(b[:, APN0 : APN0 + M].sum() for b in blocks)
    p3 = sum(b[:, P30 : P30 + NCOL].sum() for b in blocks)
    ls = sum(b[:, LS0 : LS0 + NCOL].sum() for b in blocks)
    e1 = sum(b[:, E10 : E10 + NCOL].sum() for b in blocks)
    e2 = sum(b[:, E20 : E20 + NCOL].sum() for b in blocks)
    if not v["apn_on_act"]:
        apn = -apn
    if v["ap_scale"]:
        apn *= 1.1

    # host-side terms (tiny inputs only)
    w = (mask_lightresp == 0).astype(np.float64)
    x = Jmax25.astype(np.float64)
    y = Vcmax25.astype(np.float64)
    nw = w.sum()
    if nw > 0:
        my = (w * y).sum() / nw
        mx = (w * x).sum() / nw
        vy = (y - my) * w
        vx = (x - mx) * w
        denom = np.sqrt((vx * vx).sum()) * np.sqrt((vy * vy).sum())
        cost = (vx * vy).sum() / denom if denom != 0.0 else np.nan
    else:
        cost = np.nan
    if np.isnan(cost):
        cost = 0.0
    cost = min(cost, TARGET_R)

    relu = lambda z: np.maximum(z, 0.0)
    loss = mse * 10.0 / N
    loss += TARGET_R - cost
    loss += relu(-Rd25.astype(np.float64)).sum()
    loss += relu(-dHa_Vcmax.astype(np.float64)).sum() * 10.0
    loss += relu(-dHa_Jmax.astype(np.float64)).sum()
    loss += relu(-dHa_TPU.astype(np.float64)).sum()
    loss += relu(KELVIN - Topt_Vcmax.astype(np.float64)).sum()
    loss += relu(KELVIN - Topt_Jmax.astype(np.float64)).sum()
    loss += relu(KELVIN - Topt_TPU.astype(np.float64)).sum()
    loss += apn
    loss += e1 * 0.15
    loss += e2
    loss += p3
    loss += ls

    return np.asarray(loss, dtype=np.float32)
